# revision 11
# baseline (speedup 1.0000x reference)
"""Additive attention (d2l-style) on 8 Trainium2 NeuronCores — low-rank scores.

reference math per batch b (B=8, Q=256, K=512, D=256, H=128):
    scores[q, k] = sum_h W_v[h] * tanh(qf[h,q] + kf[h,k])
    attn = softmax_k(scores), masked to k < valid_length[b]
    out  = attn @ value

The baseline evaluated tanh on all H*Q*K elements (8.9M/core) on the ACT
engine at 1 elem/cycle/lane — a hard ~60us floor.  This kernel removes the
bulk tanh entirely via a numerically-optimal separable expansion

    tanh(x + y) ~= sum_t f_t(x) g_t(y),   t = 0..R-1  (R = 8)

obtained from a Gaussian-weighted eigendecomposition of tanh(x+y) on a grid
(qf, kf ~ N(0,1); relative residual of the dropped terms ~2e-3).  Scores
then become a plain PE matmul with contraction dim R*H = 1024:

    scoresT[k, q] = sum_{t,h} G[(t,h),k] * U[(t,h),q]
    U[(t,h),q] = wv[h] f_t(qf[h,q]),  G[(t,h),k] = g_t(kf[h,k])

U and G are tiny elementwise feature maps of the O((Q+K)*H) projections and
are host-prepared (like the baseline's host-side q/k projections); the
device does all O(Q*K) work on PE: scores matmul -> exp (the only ACT op)
-> E^T @ [V | 1] with a ones-column appended to V so the softmax denominator
falls out of the same matmul -> row scale by its reciprocal.

Sharding: core b = batch b (data-parallel over B, per the hint).  Keys are
processed in NCH=4 chunks of 128 (= max ceil(K/128)), giving one fixed
instruction stream for every core and any valid_length: pad chunks carry
zeroed V/ones columns (contribute exactly 0 to numerator and denominator)
and duplicated-finite G (so exp never sees garbage).  scoresT lands in PSUM
already transposed [k, q], so exp output feeds the EV matmul directly with
no transposes anywhere.  No masking instructions: keys >= L have zeroed G
columns and zeroed V rows host-side.
"""

import sys
from contextlib import ExitStack

if "/opt/trn_rl_repo" not in sys.path:
    sys.path.insert(0, "/opt/trn_rl_repo")

import numpy as np

B, Q, K, D, H, V = 8, 256, 512, 256, 128, 256
NCORES = 8
R = 8          # separable rank of tanh(x+y)
NCH = K // 128  # key chunks per core (uniform; pads are data, not code)
A_LIM = 8.0
NGRID = 1601

_NC_CACHE = None
_BASIS = None
_LAST_RESULTS = None


def _basis():
    """Gaussian-weighted separable expansion tanh(x+y) ~= sum_t f_t(x)g_t(y)."""
    global _BASIS
    if _BASIS is None:
        x = np.linspace(-A_LIM, A_LIM, NGRID)
        w = np.exp(-0.5 * x**2) / np.sqrt(2 * np.pi) + 1e-4
        sw = np.sqrt(w)
        Aw = sw[:, None] * np.tanh(x[:, None] + x[None, :]) * sw[None, :]
        lam, phi = np.linalg.eigh(Aw)  # symmetric kernel
        idx = np.argsort(-np.abs(lam))[:R]
        lam, phi = lam[idx], phi[:, idx]
        ftab = phi * np.sqrt(np.abs(lam))[None, :] / sw[:, None]
        gtab = ftab * np.sign(lam)[None, :]
        _BASIS = (x, ftab, gtab)
    return _BASIS


def _build():
    from concourse import bacc, mybir, tile

    f32 = mybir.dt.float32
    bf16 = mybir.dt.bfloat16

    nc = bacc.Bacc(
        "TRN2",
        target_bir_lowering=False,
        debug=False,
        enable_asserts=False,
        num_devices=NCORES,
    )

    # inputs packed into 3 contiguous transfers (big per-partition lines ->
    # ~340 GB/s vs ~130 for many small-line DMAs), ordered by compute need:
    #   pk0 = U | G0 | V0,  pk1 = G1 | V1 | G2 | V2,  pk2 = G3 | V3
    GW, VW = R * 128, V + 1
    PK = [R * Q + GW + VW, 2 * (GW + VW), GW + VW]
    pk_d = [
        nc.dram_tensor(f"pk{i}", [128, PK[i]], bf16, kind="ExternalInput")
        for i in range(3)
    ]
    out_d = nc.dram_tensor("out", [2, 128, V + 1], bf16, kind="ExternalOutput")

    Exp = mybir.ActivationFunctionType.Exp

    with tile.TileContext(nc) as tc, ExitStack() as ctx:
        sb = ctx.enter_context(tc.tile_pool(name="sb", bufs=1))
        sc_ps = ctx.enter_context(tc.tile_pool(name="sc_ps", bufs=2, space="PSUM"))
        o_ps = ctx.enter_context(tc.tile_pool(name="o_ps", bufs=1, space="PSUM"))
        j_ps = ctx.enter_context(tc.tile_pool(name="j_ps", bufs=1, space="PSUM"))

        # exp table preload off the critical path
        warm = sb.tile([1, 1], f32, tag="warm")
        nc.vector.memset(warm[:, :], 0.0)
        nc.scalar.activation(warm[:, :], warm[:, :], Exp)

        # PE p-state warmup: ~1.9us of junk matmuls during the DMA lead-in
        # ramps the PE clock toward 2.4 GHz before the first real matmul
        junk = sb.tile([128, 512], bf16, tag="junk")
        nc.vector.memset(junk[:, :], 0.0)
        jp = j_ps.tile([128, 512], f32, tag="jp")
        for i in range(4):
            nc.tensor.matmul(
                jp[:, :], junk[:, :128], junk[:, :], start=(i == 0),
                stop=(i == 3),
            )

        pk_t = [
            sb.tile([128, PK[i]], bf16, tag=f"pk{i}", name=f"pk{i}")
            for i in range(3)
        ]
        for i in range(3):
            nc.sync.dma_start(pk_t[i][:, :], pk_d[i][:, :])

        # (pack tile, column offset) of each logical piece
        g_loc = [
            (pk_t[0], R * Q),
            (pk_t[1], 0),
            (pk_t[1], GW + VW),
            (pk_t[2], 0),
        ]
        v_loc = [
            (pk_t[0], R * Q + GW),
            (pk_t[1], GW),
            (pk_t[1], 2 * GW + VW),
            (pk_t[2], GW),
        ]

        def g_sl(c, t):
            tile_, off = g_loc[c]
            return tile_[:, off + t * 128 : off + (t + 1) * 128]

        def v_sl(c):
            tile_, off = v_loc[c]
            return tile_[:, off : off + VW]

        def u_sl(t):
            return pk_t[0][:, t * Q : (t + 1) * Q]

        o_tiles = [o_ps.tile([128, V + 1], f32, tag=f"o{h2}", name=f"o{h2}") for h2 in range(2)]

        def emit_scores(c):
            sc = sc_ps.tile([128, Q], f32, tag="sc")
            for t in range(R):
                nc.tensor.matmul(
                    sc[:, :],
                    g_sl(c, t),
                    u_sl(t),
                    start=(t == 0),
                    stop=(t == R - 1),
                )
            et = sb.tile([128, Q], bf16, tag=f"et{c}")
            nc.scalar.activation(et[:, :], sc[:, :], Exp)
            return et

        def emit_ev(c, et):
            for h2 in range(2):
                nc.tensor.matmul(
                    o_tiles[h2][:, :],
                    et[:, h2 * 128 : (h2 + 1) * 128],
                    v_sl(c),
                    start=(c == 0),
                    stop=(c == NCH - 1),
                )

        # pipeline: emit scores(c+1) before EV(c) so PE never waits on ACT
        pending = None
        for c in range(NCH):
            et = emit_scores(c)
            if pending is not None:
                emit_ev(*pending)
            pending = (c, et)
        emit_ev(*pending)

        # ship raw numerator|denominator; the host divides during unshard
        for h2 in range(2):
            osb = sb.tile([128, V + 1], bf16, tag=f"osb{h2}")
            nc.vector.tensor_copy(osb[:, :], o_tiles[h2][:, :])
            nc.sync.dma_start(out_d[h2, :, :], osb[:, :])

    nc.compile()
    return nc


def _feat(tab, x, pts):
    out = np.empty(pts.shape + (R,), dtype=np.float32)
    for t in range(R):
        out[..., t] = np.interp(pts, x, tab[:, t])
    return out


def _prep_in_maps(queries, key, value, W_k, W_q, W_v, Ls):
    import ml_dtypes

    bf16 = ml_dtypes.bfloat16
    x, ftab, gtab = _basis()
    wv = W_v[0].astype(np.float32)

    # host projections (tiny, <1% of FLOPs — same as baseline)
    qf = np.einsum("hd,bqd->bqh", W_q, queries, optimize=True)
    kf = np.einsum("hd,bkd->bkh", W_k, key, optimize=True)

    in_maps = []
    for b in range(B):
        L = int(Ls[b])
        # U[h, t*Q + q] = wv[h] * f_t(qf[b,q,h])
        fq = _feat(ftab, x, qf[b])                      # [Q, H, R]
        U = (fq * wv[None, :, None]).transpose(1, 2, 0)  # [H, R, Q]
        U = np.ascontiguousarray(U.reshape(H, R * Q)).astype(bf16)

        # G[c, h, t*128 + j] = g_t(kf[b, c*128+j, h]), zero for k >= L;
        # pad chunks duplicate chunk 0 (finite scores under exp, V there is 0)
        gk = _feat(gtab, x, kf[b])                      # [K, H, R]
        gk[L:] = 0.0
        G = gk.transpose(1, 2, 0).reshape(H, R, NCH, 128)
        G = np.ascontiguousarray(G.transpose(2, 0, 1, 3)).reshape(
            NCH, H, R * 128
        )
        nreal = max(1, -(-L // 128))
        G[nreal:] = G[0]
        G = G.astype(bf16)

        # V chunks with ones column; rows >= L zeroed
        Vv = np.zeros((K, V + 1), dtype=np.float32)
        Vv[:L, :V] = value[b, :L]
        Vv[:L, V] = 1.0
        Vv = Vv.reshape(NCH, 128, V + 1).astype(bf16)

        # pack: pk0 = U|G0|V0, pk1 = G1|V1|G2|V2, pk2 = G3|V3
        pk0 = np.concatenate([U, G[0], Vv[0]], axis=1)
        pk1 = np.concatenate([G[1], Vv[1], G[2], Vv[2]], axis=1)
        pk2 = np.concatenate([G[3], Vv[3]], axis=1)
        in_maps.append({"pk0": pk0, "pk1": pk1, "pk2": pk2})
    return in_maps


def kernel(queries, key, value, W_k, W_q, W_v, valid_length):
    global _NC_CACHE, _LAST_RESULTS
    queries = np.asarray(queries, dtype=np.float32)
    key = np.asarray(key, dtype=np.float32)
    value = np.asarray(value, dtype=np.float32)
    W_k = np.asarray(W_k, dtype=np.float32)
    W_q = np.asarray(W_q, dtype=np.float32)
    W_v = np.asarray(W_v, dtype=np.float32)
    Ls = tuple(int(x) for x in np.asarray(valid_length).reshape(-1))
    assert len(Ls) == B and all(1 <= L <= K for L in Ls)

    if _NC_CACHE is None:
        _NC_CACHE = _build()
    nc = _NC_CACHE

    in_maps = _prep_in_maps(queries, key, value, W_k, W_q, W_v, Ls)

    from concourse.bass_utils import run_bass_kernel_spmd

    res = run_bass_kernel_spmd(nc, in_maps, core_ids=list(range(NCORES)))
    _LAST_RESULTS = res

    out = np.empty((B, Q, V), dtype=np.float32)
    for b in range(NCORES):
        raw = res.results[b]["out"].astype(np.float32).reshape(Q, V + 1)
        out[b] = raw[:, :V] / raw[:, V : V + 1]
    return out


# revision 16
# speedup vs baseline: 1.0394x; 1.0394x over previous
"""Additive attention (d2l-style) on 8 Trainium2 NeuronCores — low-rank scores.

reference math per batch b (B=8, Q=256, K=512, D=256, H=128):
    scores[q, k] = sum_h W_v[h] * tanh(qf[h,q] + kf[h,k])
    attn = softmax_k(scores), masked to k < valid_length[b]
    out  = attn @ value

The baseline evaluated tanh on all H*Q*K elements (8.9M/core) on the ACT
engine at 1 elem/cycle/lane — a hard ~60us floor.  This kernel removes the
bulk tanh entirely via a numerically-optimal separable expansion

    tanh(x + y) ~= sum_t f_t(x) g_t(y),   t = 0..R-1  (R = 8)

obtained from a Gaussian-weighted eigendecomposition of tanh(x+y) on a grid
(qf, kf ~ N(0,1); relative residual of the dropped terms ~2e-3).  Scores
then become a plain PE matmul with contraction dim R*H = 1024:

    scoresT[k, q] = sum_{t,h} G[(t,h),k] * U[(t,h),q]
    U[(t,h),q] = wv[h] f_t(qf[h,q]),  G[(t,h),k] = g_t(kf[h,k])

U and G are tiny elementwise feature maps of the O((Q+K)*H) projections and
are host-prepared (like the baseline's host-side q/k projections); the
device does all O(Q*K) work on PE: scores matmul -> exp (the only ACT op)
-> E^T @ [V | 1] with a ones-column appended to V so the softmax denominator
falls out of the same matmul -> row scale by its reciprocal.

Sharding: core b = batch b (data-parallel over B, per the hint).  Keys are
processed in NCH=4 chunks of 128 (= max ceil(K/128)), giving one fixed
instruction stream for every core and any valid_length: pad chunks carry
zeroed V/ones columns (contribute exactly 0 to numerator and denominator)
and duplicated-finite G (so exp never sees garbage).  scoresT lands in PSUM
already transposed [k, q], so exp output feeds the EV matmul directly with
no transposes anywhere.  No masking instructions: keys >= L have zeroed G
columns and zeroed V rows host-side.
"""

import sys
from contextlib import ExitStack

if "/opt/trn_rl_repo" not in sys.path:
    sys.path.insert(0, "/opt/trn_rl_repo")

import numpy as np

B, Q, K, D, H, V = 8, 256, 512, 256, 128, 256
NCORES = 8
R = 8          # separable rank of tanh(x+y)
NCH = K // 128  # key chunks per core (uniform; pads are data, not code)
A_LIM = 8.0
NGRID = 1601

_NC_CACHE = None
_BASIS = None
_LAST_RESULTS = None


def _basis():
    """Gaussian-weighted separable expansion tanh(x+y) ~= sum_t f_t(x)g_t(y)."""
    global _BASIS
    if _BASIS is None:
        x = np.linspace(-A_LIM, A_LIM, NGRID)
        w = np.exp(-0.5 * x**2) / np.sqrt(2 * np.pi) + 1e-4
        sw = np.sqrt(w)
        Aw = sw[:, None] * np.tanh(x[:, None] + x[None, :]) * sw[None, :]
        lam, phi = np.linalg.eigh(Aw)  # symmetric kernel
        idx = np.argsort(-np.abs(lam))[:R]
        lam, phi = lam[idx], phi[:, idx]
        ftab = phi * np.sqrt(np.abs(lam))[None, :] / sw[:, None]
        gtab = ftab * np.sign(lam)[None, :]
        _BASIS = (x, ftab, gtab)
    return _BASIS


def _build():
    from concourse import bacc, mybir, tile

    f32 = mybir.dt.float32
    bf16 = mybir.dt.bfloat16

    nc = bacc.Bacc(
        "TRN2",
        target_bir_lowering=False,
        debug=False,
        enable_asserts=False,
        num_devices=NCORES,
    )

    f8 = mybir.dt.float8e4

    # components t0,t1 in bf16; t2..7 in fp8-e4m3 (halves bytes; enables
    # DoubleRow paired matmuls at 0.5 cyc/row).  Inputs packed into 6
    # contiguous transfers (one per dtype per stage) in compute-need order:
    #   pb0 = U_bf|G0_bf|V0   p80 = U_8|G0_8
    #   pb1 = G1_bf|V1|G2_bf|V2   p81 = G1_8|G2_8
    #   pb2 = G3_bf|V3   p82 = G3_8
    VW = V + 1
    UBW, U8W, GBW, G8W = 2 * Q, 6 * Q, 2 * 128, 6 * 128
    pb_d = [
        nc.dram_tensor("pb0", [128, UBW + GBW + VW], bf16, kind="ExternalInput"),
        nc.dram_tensor("pb1", [128, 2 * (GBW + VW)], bf16, kind="ExternalInput"),
        nc.dram_tensor("pb2", [128, GBW + VW], bf16, kind="ExternalInput"),
    ]
    p8_d = [
        nc.dram_tensor("p80", [128, U8W + G8W], f8, kind="ExternalInput"),
        nc.dram_tensor("p81", [128, 2 * G8W], f8, kind="ExternalInput"),
        nc.dram_tensor("p82", [128, G8W], f8, kind="ExternalInput"),
    ]
    out_d = nc.dram_tensor("out", [2, 128, V + 1], bf16, kind="ExternalOutput")

    Exp = mybir.ActivationFunctionType.Exp

    with tile.TileContext(nc) as tc, ExitStack() as ctx:
        sb = ctx.enter_context(tc.tile_pool(name="sb", bufs=1))
        sc_ps = ctx.enter_context(tc.tile_pool(name="sc_ps", bufs=2, space="PSUM"))
        o_ps = ctx.enter_context(tc.tile_pool(name="o_ps", bufs=1, space="PSUM"))
        j_ps = ctx.enter_context(tc.tile_pool(name="j_ps", bufs=1, space="PSUM"))

        # exp table preload off the critical path
        warm = sb.tile([1, 1], f32, tag="warm")
        nc.vector.memset(warm[:, :], 0.0)
        nc.scalar.activation(warm[:, :], warm[:, :], Exp)

        # PE p-state warmup junk during the DMA lead-in
        junk = sb.tile([128, 512], bf16, tag="junk")
        nc.vector.memset(junk[:, :], 0.0)
        jp = j_ps.tile([128, 512], f32, tag="jp")
        for i in range(2):
            nc.tensor.matmul(
                jp[:, :], junk[:, :128], junk[:, :], start=(i == 0),
                stop=(i == 1),
            )

        pb_t = [
            sb.tile(list(pb_d[i].shape), bf16, tag=f"pb{i}", name=f"pb{i}")
            for i in range(3)
        ]
        p8_t = [
            sb.tile(list(p8_d[i].shape), f8, tag=f"p8{i}", name=f"p8{i}")
            for i in range(3)
        ]
        for i in range(3):
            nc.sync.dma_start(pb_t[i][:, :], pb_d[i][:, :])
            nc.sync.dma_start(p8_t[i][:, :], p8_d[i][:, :])

        # (tile, column offset) of each logical piece
        gb_loc = [(pb_t[0], UBW), (pb_t[1], 0), (pb_t[1], GBW + VW), (pb_t[2], 0)]
        g8_loc = [(p8_t[0], U8W), (p8_t[1], 0), (p8_t[1], G8W), (p8_t[2], 0)]
        v_loc = [
            (pb_t[0], UBW + GBW),
            (pb_t[1], GBW),
            (pb_t[1], 2 * GBW + VW),
            (pb_t[2], GBW),
        ]

        def g_bf(c, t):
            tile_, off = gb_loc[c]
            return tile_[:, off + t * 128 : off + (t + 1) * 128]

        def g_8pair(c, i):
            tile_, off = g8_loc[c]
            sl = tile_[:, off + i * 256 : off + (i + 1) * 256]
            return sl.rearrange("p (two f) -> p two f", two=2)

        def u_bf(t):
            return pb_t[0][:, t * Q : (t + 1) * Q]

        def u_8pair(i):
            sl = p8_t[0][:, 2 * i * Q : 2 * (i + 1) * Q]
            return sl.rearrange("p (two f) -> p two f", two=2)

        def v_sl(c):
            tile_, off = v_loc[c]
            return tile_[:, off : off + VW]

        o_tiles = [o_ps.tile([128, V + 1], f32, tag=f"o{h2}", name=f"o{h2}") for h2 in range(2)]

        DR = mybir.MatmulPerfMode.DoubleRow

        def emit_scores(c):
            sc = sc_ps.tile([128, Q], f32, tag="sc")
            for t in range(2):
                nc.tensor.matmul(
                    sc[:, :], g_bf(c, t), u_bf(t), start=(t == 0), stop=False
                )
            for i in range(3):
                nc.tensor.matmul(
                    sc[:, :],
                    g_8pair(c, i),
                    u_8pair(i),
                    start=False,
                    stop=(i == 2),
                    perf_mode=DR,
                )
            et = sb.tile([128, Q], bf16, tag=f"et{c}")
            nc.scalar.activation(et[:, :], sc[:, :], Exp)
            return et

        def emit_ev(c, et):
            for h2 in range(2):
                nc.tensor.matmul(
                    o_tiles[h2][:, :],
                    et[:, h2 * 128 : (h2 + 1) * 128],
                    v_sl(c),
                    start=(c == 0),
                    stop=(c == NCH - 1),
                )

        # pipeline: emit scores(c+1) before EV(c) so PE never waits on ACT
        pending = None
        for c in range(NCH):
            et = emit_scores(c)
            if pending is not None:
                emit_ev(*pending)
            pending = (c, et)
        emit_ev(*pending)

        # ship raw numerator|denominator; the host divides during unshard
        for h2 in range(2):
            osb = sb.tile([128, V + 1], bf16, tag=f"osb{h2}")
            nc.vector.tensor_copy(osb[:, :], o_tiles[h2][:, :])
            nc.sync.dma_start(out_d[h2, :, :], osb[:, :])

    nc.compile()
    return nc


def _feat(tab, x, pts):
    out = np.empty(pts.shape + (R,), dtype=np.float32)
    for t in range(R):
        out[..., t] = np.interp(pts, x, tab[:, t])
    return out


def _prep_in_maps(queries, key, value, W_k, W_q, W_v, Ls):
    import ml_dtypes

    bf16 = ml_dtypes.bfloat16
    f8 = ml_dtypes.float8_e4m3fn
    x, ftab, gtab = _basis()
    wv = W_v[0].astype(np.float32)

    # host projections (tiny, <1% of FLOPs — same as baseline)
    qf = np.einsum("hd,bqd->bqh", W_q, queries, optimize=True)
    kf = np.einsum("hd,bkd->bkh", W_k, key, optimize=True)

    in_maps = []
    for b in range(B):
        L = int(Ls[b])
        # U[h, t*Q + q] = wv[h] * f_t(qf[b,q,h])
        fq = _feat(ftab, x, qf[b])                      # [Q, H, R]
        U = (fq * wv[None, :, None]).transpose(1, 2, 0)  # [H, R, Q]
        U = np.ascontiguousarray(U.reshape(H, R * Q))
        U_bf = U[:, : 2 * Q].astype(bf16)
        U_8 = U[:, 2 * Q :].astype(f8)

        # G[c, h, t*128 + j] = g_t(kf[b, c*128+j, h]), zero for k >= L;
        # pad chunks duplicate chunk 0 (finite scores under exp, V there is 0)
        gk = _feat(gtab, x, kf[b])                      # [K, H, R]
        gk[L:] = 0.0
        G = gk.transpose(1, 2, 0).reshape(H, R, NCH, 128)
        G = np.ascontiguousarray(G.transpose(2, 0, 1, 3)).reshape(
            NCH, H, R * 128
        )
        nreal = max(1, -(-L // 128))
        G[nreal:] = G[0]
        G_bf = G[:, :, : 2 * 128].astype(bf16)
        G_8 = G[:, :, 2 * 128 :].astype(f8)

        # V chunks with ones column; rows >= L zeroed
        Vv = np.zeros((K, V + 1), dtype=np.float32)
        Vv[:L, :V] = value[b, :L]
        Vv[:L, V] = 1.0
        Vv = Vv.reshape(NCH, 128, V + 1).astype(bf16)

        in_maps.append({
            "pb0": np.concatenate([U_bf, G_bf[0], Vv[0]], axis=1),
            "p80": np.concatenate([U_8, G_8[0]], axis=1),
            "pb1": np.concatenate([G_bf[1], Vv[1], G_bf[2], Vv[2]], axis=1),
            "p81": np.concatenate([G_8[1], G_8[2]], axis=1),
            "pb2": np.concatenate([G_bf[3], Vv[3]], axis=1),
            "p82": G_8[3],
        })
    return in_maps


def kernel(queries, key, value, W_k, W_q, W_v, valid_length):
    global _NC_CACHE, _LAST_RESULTS
    queries = np.asarray(queries, dtype=np.float32)
    key = np.asarray(key, dtype=np.float32)
    value = np.asarray(value, dtype=np.float32)
    W_k = np.asarray(W_k, dtype=np.float32)
    W_q = np.asarray(W_q, dtype=np.float32)
    W_v = np.asarray(W_v, dtype=np.float32)
    Ls = tuple(int(x) for x in np.asarray(valid_length).reshape(-1))
    assert len(Ls) == B and all(1 <= L <= K for L in Ls)

    if _NC_CACHE is None:
        _NC_CACHE = _build()
    nc = _NC_CACHE

    in_maps = _prep_in_maps(queries, key, value, W_k, W_q, W_v, Ls)

    from concourse.bass_utils import run_bass_kernel_spmd

    res = run_bass_kernel_spmd(nc, in_maps, core_ids=list(range(NCORES)))
    _LAST_RESULTS = res

    out = np.empty((B, Q, V), dtype=np.float32)
    for b in range(NCORES):
        raw = res.results[b]["out"].astype(np.float32).reshape(Q, V + 1)
        out[b] = raw[:, :V] / raw[:, V : V + 1]
    return out
